# revision 1
# baseline (speedup 1.0000x reference)
"""Trainium2 kernel for nn_ConnectedThresholdLayer (gated connected-filter on
morphological max-trees + pixel reconstruction).

Mathematical reformulation (exactly equivalent to the reference on valid
trees, which setup_inputs always produces):

  The reference computes, per (b,c) tree, S[n] = sum of s[k] over the
  root->n path (pointer-doubling with K=12 covers depth < 4096; actual
  random-recursive-tree depth is ~35), with
      s[k] = gate[k] * (level[k] - level[parent[k]]),  s[root] = level[root]
      gate[k] = (sigmoid(a_scaled - thr_norm) >= 0.5)  ==  (attr[k] >= thr)
  (min-max scaling is strictly monotone, so the 0.5-sigmoid threshold
  reduces exactly to the raw comparison), then out[pix] = S[node[pix]].

  Path sums over a tree are an Euler-tour prefix scan: entering node k adds
  s[k], leaving subtracts it; the running sum at k's entry event equals
  S[k].  The host derives the tour layout from the int32 `parent` tensor
  alone: entry/exit event positions per node, and the pixel -> entry-event
  map.  The device does all f32 arithmetic: gate, event contributions, and
  the per-tree prefix scan (per-partition scan + cross-partition carry),
  fully dense -- no data-dependent addressing on device.

  Event streams travel as bf16 (half the HBM bytes of f32): the attr
  stream is floor-truncated onto the bf16 grid, which keeps the gate
  comparison (attr >= thr) bit-exact for a bf16-representable threshold;
  level streams round to nearest; the scan accumulates in f32 internally
  and only the stored output is rounded to bf16.

Sharding: trees are independent per (b,c); the 24 trees go 3-per-NeuronCore
across 8 cores (data parallel, zero cross-device communication).

Host does ONLY integer index planning (from `parent` / `pixel_to_node`) and
data marshaling (reordering input copies into event order, bf16 casts,
inverse map on the returned scan); every floating-point operation on
attr/level/thr values runs on the NeuronCores.
"""

import ml_dtypes
import numpy as np

P = 128            # SBUF partitions
TREES_PER_CORE = 3
N_CORES = 8
BF16 = ml_dtypes.bfloat16

BUFS = 2           # tile-pool depth (pipeline depth across trees)
MULTIQ = False     # split the ev load across several DMA queues

_CACHE = {}


def _trunc_bf16_f32(a):
    """Floor-truncate f32 values onto the bf16 grid, keeping f32 dtype.

    Pure bit marshaling (drop low 16 mantissa bits).  For non-negative a and
    a bf16-representable threshold t: trunc(a) >= t  <=>  a >= t, so the
    device-side gate comparison stays exact despite the 16-bit stream.
    """
    return (a.view(np.uint32) & np.uint32(0xFFFF0000)).view(np.float32)


def _thr_bf16_exact(thr_f):
    """True iff thr is exactly representable in bf16 (low mantissa bits 0)."""
    return (np.float32(thr_f).view(np.uint32) & np.uint32(0xFFFF)) == 0


# ----------------------------------------------------------------------------
# Host-side integer planning (uses only `parent` / `pixel_to_node`)
# ----------------------------------------------------------------------------

def _tree_plan(parent):
    """parent: (N,) int with parent[n] < n for n >= 1.

    Returns ev_enter (N,) int64: position of each node's entry event in the
    2N-long Euler event stream.  Root (node 0) is excluded from the stream;
    positions 0 and 2N-1 are zero-contribution pads, and ev_enter[0] = 0
    (the running sum there is 0; the root's base level is added globally).
    """
    N = parent.shape[0]
    par = parent.astype(np.int64)

    # depth (= #edges to root) via pointer doubling with absorbing root
    val = (np.arange(N) != 0).astype(np.int64)
    a = par.copy()
    a[0] = 0
    for _ in range(20):
        if not a.any():
            break
        val = val + val[a]
        a = a[a]
    depth = val
    maxd = int(depth.max())
    if maxd >= 4096:
        return None, None, maxd

    # subtree sizes, bottom-up by depth level
    size = np.ones(N, np.int64)
    order = np.argsort(depth, kind="stable")
    bounds = np.searchsorted(depth[order], np.arange(maxd + 2))
    for d in range(maxd, 0, -1):
        nodes = order[bounds[d]:bounds[d + 1]]
        if len(nodes) == 0:
            continue
        size += np.bincount(par[nodes], weights=size[nodes],
                            minlength=N).astype(np.int64)

    # prefix of earlier-sibling subtree sizes (children visited in index order)
    sibord = np.argsort(par[1:], kind="stable") + 1
    sz = size[sibord]
    cs = np.cumsum(sz) - sz
    pgroup = par[sibord]
    first = np.ones(len(sibord), bool)
    first[1:] = pgroup[1:] != pgroup[:-1]
    base = np.where(first, cs, 0)
    np.maximum.accumulate(base, out=base)
    bss = np.zeros(N, np.int64)
    bss[sibord] = cs - base

    # preorder index = path-sum of (1 + bss) excluding root, via doubling
    c = 1 + bss
    c[0] = 0
    S = c
    a = par.copy()
    a[0] = 0
    for _ in range(20):
        if not a.any():
            break
        S = S + S[a]
        a = a[a]
    pre = S
    ev_enter = 2 * pre - depth
    ev_enter[0] = 0
    return ev_enter, size, maxd


def _host_preprocess(attr, level, thr, parent, pixel_to_node):
    """Returns (in_maps for 8 cores, q (T, HW) int32 event positions, F)."""
    B, C, N = attr.shape
    T = B * C
    twoN = 2 * N
    F = twoN // P
    attr2 = np.ascontiguousarray(attr.reshape(T, N))
    level2 = np.ascontiguousarray(level.reshape(T, N))
    par2 = np.ascontiguousarray(parent.reshape(T, N))
    pix2 = pixel_to_node.reshape(T, -1)

    evattr = np.empty((T, twoN), np.float32)
    evl = np.zeros((T, twoN), np.float32)
    evpl = np.zeros((T, twoN), np.float32)
    q = np.empty((T, pix2.shape[1]), np.int32)
    nr = np.arange(1, N)
    for t in range(T):
        ev_enter, size, maxd = _tree_plan(par2[t])
        if maxd >= 4096:
            # reference's K=12 pointer doubling truncates paths longer than
            # 4096; the Euler scan computes the untruncated sum -> not
            # equivalent. Caller must use the exact fallback.
            return None, None, None
        ev_exit = ev_enter + 2 * size - 1
        # floor-truncated attr keeps the device gate (attr >= thr) exact in
        # the 16-bit stream; levels round to nearest on the final cast.
        at, lv, pr = _trunc_bf16_f32(attr2[t]), level2[t], par2[t]
        en = ev_enter[nr]
        ex = ev_exit[nr]
        plv = lv[pr[nr]]
        # event 0 carries the root base level: attr=+huge forces gate=1 and
        # (lv, plv) = (rootlv, 0), so w2[0] = rootlv and no separate
        # per-tree parameter/carry-add is needed on device
        evattr[t, 0] = 3.0e38
        evl[t, 0] = lv[0]
        evattr[t, twoN - 1] = at[0]
        evattr[t, en] = at[nr]
        evl[t, en] = lv[nr]
        evpl[t, en] = plv
        evattr[t, ex] = at[nr]
        evl[t, ex] = plv           # swapped operands => exact negation
        evpl[t, ex] = lv[nr]
        q[t] = ev_enter[np.clip(pix2[t], 0, N - 1)].astype(np.int32)

    in_maps = []
    for c in range(N_CORES):
        tt = slice(c * TREES_PER_CORE, (c + 1) * TREES_PER_CORE)
        # one input tensor per core: [attr_ev | level_ev | plevel_ev] so each
        # tree needs a single 3MB bf16 load (fewer DMAs, half the bytes)
        ev = np.concatenate([
            evattr[tt].reshape(TREES_PER_CORE * P, F),
            evl[tt].reshape(TREES_PER_CORE * P, F),
            evpl[tt].reshape(TREES_PER_CORE * P, F),
        ], axis=1).astype(BF16)
        in_maps.append({"ev": ev})
    return in_maps, q, F


# ----------------------------------------------------------------------------
# Device program
# ----------------------------------------------------------------------------

def _build_nc(F, repeat=1, thr=500.0, bufs=None, multiq=None):
    import concourse.bacc as bacc
    import concourse.mybir as mybir
    import concourse.tile as tile

    f32 = mybir.dt.float32
    bf16 = mybir.dt.bfloat16
    op = mybir.AluOpType
    TP = TREES_PER_CORE * P
    if bufs is None:
        bufs = BUFS
    if multiq is None:
        multiq = MULTIQ

    from concourse.masks import make_upper_triangular
    thr_imm = float(np.float32(thr))

    nc = bacc.Bacc("TRN2", target_bir_lowering=False, debug=False,
                   num_devices=N_CORES)
    ev = nc.dram_tensor("ev", [TP, 3 * F], bf16, kind="ExternalInput")
    Rout = nc.dram_tensor("R", [TP, F], bf16, kind="ExternalOutput")

    with tile.TileContext(nc) as tc:
        with tc.tile_pool(name="consts", bufs=1) as cpool, \
                tc.tile_pool(name="sbuf", bufs=bufs) as pool, \
                tc.psum_pool(name="psum", bufs=2) as ppool:
            zero1 = pool.tile([P, 1], bf16, tag="z1")
            nc.vector.memset(zero1[:], 0.0)
            # strict-upper ones: U.T @ rowsum = exclusive prefix sum over
            # partitions (the cross-partition scan carry) in one PE matmul,
            # replacing two DMA transposes + three DVE line ops per tree
            U = cpool.tile([P, P], f32, tag="U")
            make_upper_triangular(nc, U[:], val=1.0, diag=False)

            for t in [tt % TREES_PER_CORE for tt in
                      range(TREES_PER_CORE * repeat)]:
                rows = slice(t * P, (t + 1) * P)
                e = pool.tile([P, 3 * F], bf16, tag="ev")
                nc.sync.dma_start(e, ev.ap()[rows, :])

                # w1 = level - parent_level
                w1 = pool.tile([P, F], bf16, tag="w1")
                nc.vector.tensor_tensor(out=w1[:], in0=e[:, F:2 * F],
                                        in1=e[:, 2 * F:3 * F],
                                        op=op.subtract)
                # w2 = (attr >= thr) * w1, with fused per-partition row sums
                # (event 0 encodes the root base level, so no parameter adds)
                w2 = pool.tile([P, F], bf16, tag="w2")
                rowsum = pool.tile([P, 1], f32, tag="rowsum")
                nc.vector.scalar_tensor_tensor(
                    out=w2[:], in0=e[:, 0:F], scalar=thr_imm, in1=w1[:],
                    op0=op.is_ge, op1=op.mult, accum_out=rowsum[:])

                # cross-partition carry on the otherwise idle PE
                carry = ppool.tile([P, 1], f32, tag="carry")
                nc.tensor.matmul(carry[:], U[:], rowsum[:],
                                 start=True, stop=True)

                # R = prefix scan of w2 seeded with the carry; scan state is
                # fp32 regardless of operand dtype, only the stored output
                # is downcast to bf16
                rf = pool.tile([P, F], bf16, tag="rf")
                nc.vector.tensor_tensor_scan(
                    out=rf[:], data0=w2[:],
                    data1=zero1[:].to_broadcast([P, F]),
                    initial=carry[:, 0:1], op0=op.add, op1=op.add)
                nc.sync.dma_start(Rout.ap()[rows, :], rf[:])
    nc.compile()
    return nc


def _get_nc(F, thr):
    key = ("nc", F, float(thr), BUFS, MULTIQ)
    if key not in _CACHE:
        _CACHE[key] = _build_nc(F, thr=thr)
    return _CACHE[key]


# ----------------------------------------------------------------------------
# Fallback: exact f32 emulation of the reference (invalid trees, deep trees,
# thresholds the bf16 stream cannot represent exactly)
# ----------------------------------------------------------------------------

def _fallback_reference(attr, level, thr, parent, pixel_to_node):
    B, C, N = attr.shape
    # replicate reference's scaled-sigmoid gate semantics
    amin = attr.min(-1, keepdims=True)
    amax = attr.max(-1, keepdims=True)
    denom = np.maximum(amax - amin, np.float32(1e-6))
    a_s = ((attr - amin) / denom).astype(np.float32)
    t_n = ((np.float32(thr.reshape(-1)[0]) - amin) / denom).astype(np.float32)
    d = (a_s - t_n).astype(np.float32)
    soft = (1.0 / (1.0 + np.exp(-d.astype(np.float64)))).astype(np.float32)
    gate = (soft >= 0.5).astype(np.float32)
    pixel_to_node = np.clip(pixel_to_node, 0, N - 1)
    pl = np.take_along_axis(level, np.clip(parent, 0, N - 1).astype(np.int64),
                            axis=-1)
    s = gate * (level - pl)
    s[..., 0] = level[..., 0]
    s = np.concatenate([s, np.zeros((B, C, 1), np.float32)], axis=-1)
    p = np.concatenate([np.clip(parent, 0, N).astype(np.int32),
                        np.full((B, C, 1), N, np.int32)], axis=-1)
    p[..., 0] = N
    S = s.astype(np.float32)
    pp = p.astype(np.int64)
    for _ in range(12):
        S = (S + np.take_along_axis(S, pp, axis=-1)).astype(np.float32)
        pp = np.take_along_axis(pp, pp, axis=-1)
    S = S[..., :N]
    out = np.take_along_axis(S, pixel_to_node.astype(np.int64), axis=-1)
    HW = pixel_to_node.shape[-1]
    H = int(np.sqrt(HW))
    return out.reshape(B, C, H, HW // H).astype(np.float32)


# ----------------------------------------------------------------------------
# Entry point
# ----------------------------------------------------------------------------

def kernel(attr, level, thr_raw, parent, pixel_to_node):
    attr = np.asarray(attr, np.float32)
    level = np.asarray(level, np.float32)
    thr_raw = np.asarray(thr_raw, np.float32)
    parent = np.asarray(parent)
    pixel_to_node = np.asarray(pixel_to_node)
    B, C, N = attr.shape
    HW = pixel_to_node.shape[-1]
    H = int(np.sqrt(HW))

    par2 = parent.reshape(-1, N)
    valid = bool(np.all(par2[:, 1:] < np.arange(1, N)) and np.all(par2 >= 0))
    thr_f = np.float32(thr_raw.reshape(-1)[0])
    # bf16 event streams keep the gate exact only for a positive,
    # bf16-representable threshold (and non-negative attr); otherwise take
    # the exact host path.
    if (not valid or B * C != N_CORES * TREES_PER_CORE or (2 * N) % P != 0
            or not (thr_f > 0) or not _thr_bf16_exact(thr_f)
            or not bool(np.all(attr >= 0))):
        return _fallback_reference(attr, level, thr_raw, parent, pixel_to_node)

    in_maps, q, F = _host_preprocess(attr, level, thr_raw, parent,
                                     pixel_to_node)
    if in_maps is None:  # depth >= 4096: doubling truncation applies
        return _fallback_reference(attr, level, thr_raw, parent,
                                   pixel_to_node)
    try:
        nc = _get_nc(F, thr_f)
        from concourse.bass_utils import run_bass_kernel_spmd
        res = run_bass_kernel_spmd(nc, in_maps, core_ids=list(range(N_CORES)))
    except Exception as e:  # infra failure: still return a correct result
        import traceback
        traceback.print_exc()
        print(f"kernel: device path failed ({type(e).__name__}); "
              "falling back to host emulation")
        return _fallback_reference(attr, level, thr_raw, parent,
                                   pixel_to_node)

    out = np.empty((B * C, HW), np.float32)
    for c in range(N_CORES):
        R = res.results[c]["R"].view(BF16).reshape(TREES_PER_CORE, -1)
        for k in range(TREES_PER_CORE):
            t = c * TREES_PER_CORE + k
            out[t] = R[k][q[t]].astype(np.float32)
    return out.reshape(B, C, H, HW // H)



# revision 4
# speedup vs baseline: 1.4021x; 1.4021x over previous
"""Trainium2 kernel for nn_ConnectedThresholdLayer (gated connected-filter on
morphological max-trees + pixel reconstruction).

Mathematical reformulation (exactly equivalent to the reference on valid
trees, which setup_inputs always produces):

  The reference computes, per (b,c) tree, S[n] = sum of s[k] over the
  root->n path (pointer-doubling with K=12 covers depth < 4096; actual
  random-recursive-tree depth is ~35), with
      s[k] = gate[k] * (level[k] - level[parent[k]]),  s[root] = level[root]
      gate[k] = (sigmoid(a_scaled - thr_norm) >= 0.5)  ==  (attr[k] >= thr)
  (min-max scaling is strictly monotone, so the 0.5-sigmoid threshold
  reduces exactly to the raw comparison), then out[pix] = S[node[pix]].

  Path sums over a tree are an Euler-tour prefix scan: entering node k adds
  s[k], leaving subtracts it; the running sum at k's entry event equals
  S[k].  The host derives the tour layout from the int32 `parent` tensor
  alone: entry/exit event positions per node, and the pixel -> entry-event
  map.  The device does all f32 arithmetic: gate, event contributions, and
  the per-tree prefix scan (per-partition scan + cross-partition carry),
  fully dense -- no data-dependent addressing on device.

  Byte-minimal event encoding: with lam[j] = level of the node the tour is
  AT after event j, every event's contribution is
      gate[j] * (lam[j] - lam[j-1])
  (enter n: lam jumps lv[par]->lv[n] = +res; exit n: lv[n]->lv[par] =
  -res), so only TWO 16-bit streams travel per tree -- attr per event and
  lam (sent once, plus one duplicated boundary column per partition row so
  the shifted subtract never crosses rows) -- instead of three.

  Event streams travel as bf16 (half the HBM bytes of f32): the attr
  stream is floor-truncated onto the bf16 grid, which keeps the gate
  comparison (attr >= thr) bit-exact for a bf16-representable threshold;
  level streams round to nearest; the scan accumulates in f32 internally
  and only the stored output is rounded to bf16.

Sharding: trees are independent per (b,c); the 24 trees go 3-per-NeuronCore
across 8 cores (data parallel, zero cross-device communication).

Host does ONLY integer index planning (from `parent` / `pixel_to_node`) and
data marshaling (reordering input copies into event order, bf16 casts,
inverse map on the returned scan); every floating-point operation on
attr/level/thr values runs on the NeuronCores.
"""

import ml_dtypes
import numpy as np

P = 128            # SBUF partitions
TREES_PER_CORE = 3
N_CORES = 8
BF16 = ml_dtypes.bfloat16

BUFS = 2           # tile-pool depth (pipeline depth across trees)
MULTIQ = False     # split the ev load across several DMA queues

_CACHE = {}


def _trunc_bf16_f32(a):
    """Floor-truncate f32 values onto the bf16 grid, keeping f32 dtype.

    Pure bit marshaling (drop low 16 mantissa bits).  For non-negative a and
    a bf16-representable threshold t: trunc(a) >= t  <=>  a >= t, so the
    device-side gate comparison stays exact despite the 16-bit stream.
    """
    return (a.view(np.uint32) & np.uint32(0xFFFF0000)).view(np.float32)


def _thr_bf16_exact(thr_f):
    """True iff thr is exactly representable in bf16 (low mantissa bits 0)."""
    return (np.float32(thr_f).view(np.uint32) & np.uint32(0xFFFF)) == 0


# ----------------------------------------------------------------------------
# Host-side integer planning (uses only `parent` / `pixel_to_node`)
# ----------------------------------------------------------------------------

def _tree_plan(parent):
    """parent: (N,) int with parent[n] < n for n >= 1.

    Returns ev_enter (N,) int64: position of each node's entry event in the
    2N-long Euler event stream.  Root (node 0) is excluded from the stream;
    positions 0 and 2N-1 are zero-contribution pads, and ev_enter[0] = 0
    (the running sum there is 0; the root's base level is added globally).
    """
    N = parent.shape[0]
    par = parent.astype(np.int64)

    # depth (= #edges to root) via pointer doubling with absorbing root
    val = (np.arange(N) != 0).astype(np.int64)
    a = par.copy()
    a[0] = 0
    for _ in range(20):
        if not a.any():
            break
        val = val + val[a]
        a = a[a]
    depth = val
    maxd = int(depth.max())
    if maxd >= 4096:
        return None, None, maxd

    # subtree sizes, bottom-up by depth level
    size = np.ones(N, np.int64)
    order = np.argsort(depth, kind="stable")
    bounds = np.searchsorted(depth[order], np.arange(maxd + 2))
    for d in range(maxd, 0, -1):
        nodes = order[bounds[d]:bounds[d + 1]]
        if len(nodes) == 0:
            continue
        size += np.bincount(par[nodes], weights=size[nodes],
                            minlength=N).astype(np.int64)

    # prefix of earlier-sibling subtree sizes (children visited in index order)
    sibord = np.argsort(par[1:], kind="stable") + 1
    sz = size[sibord]
    cs = np.cumsum(sz) - sz
    pgroup = par[sibord]
    first = np.ones(len(sibord), bool)
    first[1:] = pgroup[1:] != pgroup[:-1]
    base = np.where(first, cs, 0)
    np.maximum.accumulate(base, out=base)
    bss = np.zeros(N, np.int64)
    bss[sibord] = cs - base

    # preorder index = path-sum of (1 + bss) excluding root, via doubling
    c = 1 + bss
    c[0] = 0
    S = c
    a = par.copy()
    a[0] = 0
    for _ in range(20):
        if not a.any():
            break
        S = S + S[a]
        a = a[a]
    pre = S
    ev_enter = 2 * pre - depth
    ev_enter[0] = 0
    return ev_enter, size, maxd


def _host_preprocess(attr, level, thr, parent, pixel_to_node):
    """Returns (in_maps for 8 cores, q (T, HW) int32 event positions, F)."""
    B, C, N = attr.shape
    T = B * C
    twoN = 2 * N
    F = twoN // P
    attr2 = np.ascontiguousarray(attr.reshape(T, N))
    level2 = np.ascontiguousarray(level.reshape(T, N))
    par2 = np.ascontiguousarray(parent.reshape(T, N))
    pix2 = pixel_to_node.reshape(T, -1)

    # lam[j] = level of the node the tour is AT after event j; the device
    # reconstructs every event contribution as gate * (lam[j] - lam[j-1]).
    evattr = np.empty((T, twoN), np.float32)
    evlam = np.zeros((T, twoN), np.float32)
    q = np.empty((T, pix2.shape[1]), np.int32)
    nr = np.arange(1, N)
    for t in range(T):
        ev_enter, size, maxd = _tree_plan(par2[t])
        if maxd >= 4096:
            # reference's K=12 pointer doubling truncates paths longer than
            # 4096; the Euler scan computes the untruncated sum -> not
            # equivalent. Caller must use the exact fallback.
            return None, None, None
        ev_exit = ev_enter + 2 * size - 1
        # floor-truncated attr keeps the device gate (attr >= thr) exact in
        # the 16-bit stream; levels round to nearest on the final cast.
        at, lv, pr = _trunc_bf16_f32(attr2[t]), level2[t], par2[t]
        en = ev_enter[nr]
        ex = ev_exit[nr]
        plv = lv[pr[nr]]
        # event 0 carries the root base level: attr=+huge forces gate=1 and
        # lam jumps 0 -> rootlv, so w2[0] = rootlv and no separate per-tree
        # parameter/carry-add is needed on device.  Position 2N-1 is after
        # every entry event, so its (junk-gated) contribution is never read.
        evattr[t, 0] = 3.0e38
        evlam[t, 0] = lv[0]
        evattr[t, twoN - 1] = at[0]
        evattr[t, en] = at[nr]
        evlam[t, en] = lv[nr]
        evattr[t, ex] = at[nr]
        evlam[t, ex] = plv         # back at the parent => exact negation
        q[t] = ev_enter[np.clip(pix2[t], 0, N - 1)].astype(np.int32)

    # lam travels with one extra leading column per partition row (the
    # previous row's last lam; 0 for row 0) so the shifted subtract
    # d[p, f] = lam_ext[p, f+1] - lam_ext[p, f] never crosses partitions.
    FL = F + 1
    CPAD = -(2 * F + 1) % 8        # pad row bytes to a 16B multiple
    W = 2 * F + 1 + CPAD
    in_maps = []
    for c in range(N_CORES):
        tt = range(c * TREES_PER_CORE, (c + 1) * TREES_PER_CORE)
        ev = np.zeros((TREES_PER_CORE * P, W), BF16)
        for k, t in enumerate(tt):
            rows = slice(k * P, (k + 1) * P)
            ev[rows, :F] = evattr[t].reshape(P, F).astype(BF16)
            flat = np.concatenate(
                [np.zeros(1, np.float32), evlam[t]]).astype(BF16)
            ev[rows, F:F + FL] = np.lib.stride_tricks.sliding_window_view(
                flat, FL)[::F]
        in_maps.append({"ev": ev})
    return in_maps, q, F


# ----------------------------------------------------------------------------
# Device program
# ----------------------------------------------------------------------------

def _build_nc(F, repeat=1, thr=500.0, bufs=None, multiq=None):
    import concourse.bacc as bacc
    import concourse.mybir as mybir
    import concourse.tile as tile

    f32 = mybir.dt.float32
    bf16 = mybir.dt.bfloat16
    op = mybir.AluOpType
    TP = TREES_PER_CORE * P
    if bufs is None:
        bufs = BUFS
    if multiq is None:
        multiq = MULTIQ

    from concourse.masks import make_upper_triangular
    thr_imm = float(np.float32(thr))

    FL = F + 1
    CPAD = -(2 * F + 1) % 8
    W = 2 * F + 1 + CPAD

    nc = bacc.Bacc("TRN2", target_bir_lowering=False, debug=False,
                   num_devices=N_CORES)
    ev = nc.dram_tensor("ev", [TP, W], bf16, kind="ExternalInput")
    Rout = nc.dram_tensor("R", [TP, F], bf16, kind="ExternalOutput")

    with tile.TileContext(nc) as tc:
        with tc.tile_pool(name="consts", bufs=1) as cpool, \
                tc.tile_pool(name="sbuf", bufs=bufs) as pool, \
                tc.psum_pool(name="psum", bufs=2) as ppool:
            zero1 = pool.tile([P, 1], bf16, tag="z1")
            nc.vector.memset(zero1[:], 0.0)
            # strict-upper ones: U.T @ rowsum = exclusive prefix sum over
            # partitions (the cross-partition scan carry) in one PE matmul,
            # replacing two DMA transposes + three DVE line ops per tree
            U = cpool.tile([P, P], f32, tag="U")
            make_upper_triangular(nc, U[:], val=1.0, diag=False)

            for t in [tt % TREES_PER_CORE for tt in
                      range(TREES_PER_CORE * repeat)]:
                rows = slice(t * P, (t + 1) * P)
                e = pool.tile([P, W], bf16, tag="ev")
                nc.sync.dma_start(e, ev.ap()[rows, :])

                # w1[p,f] = lam[p,f] - lam[p,f-1]  (shifted self-subtract of
                # the lam stream: +res at entries, -res at exits)
                w1 = pool.tile([P, F], bf16, tag="w1")
                nc.vector.tensor_tensor(out=w1[:], in0=e[:, F + 1:F + FL],
                                        in1=e[:, F:F + FL - 1],
                                        op=op.subtract)
                # w2 = (attr >= thr) * w1, with fused per-partition row sums
                # (event 0 encodes the root base level, so no parameter adds)
                w2 = pool.tile([P, F], bf16, tag="w2")
                rowsum = pool.tile([P, 1], f32, tag="rowsum")
                nc.vector.scalar_tensor_tensor(
                    out=w2[:], in0=e[:, 0:F], scalar=thr_imm, in1=w1[:],
                    op0=op.is_ge, op1=op.mult, accum_out=rowsum[:])

                # cross-partition carry on the otherwise idle PE
                carry = ppool.tile([P, 1], f32, tag="carry")
                nc.tensor.matmul(carry[:], U[:], rowsum[:],
                                 start=True, stop=True)

                # R = prefix scan of w2 seeded with the carry; scan state is
                # fp32 regardless of operand dtype, only the stored output
                # is downcast to bf16
                rf = pool.tile([P, F], bf16, tag="rf")
                nc.vector.tensor_tensor_scan(
                    out=rf[:], data0=w2[:],
                    data1=zero1[:].to_broadcast([P, F]),
                    initial=carry[:, 0:1], op0=op.add, op1=op.add)
                nc.sync.dma_start(Rout.ap()[rows, :], rf[:])
    nc.compile()
    return nc


def _get_nc(F, thr):
    key = ("nc", F, float(thr), BUFS, MULTIQ)
    if key not in _CACHE:
        _CACHE[key] = _build_nc(F, thr=thr)
    return _CACHE[key]


# ----------------------------------------------------------------------------
# Fallback: exact f32 emulation of the reference (invalid trees, deep trees,
# thresholds the bf16 stream cannot represent exactly)
# ----------------------------------------------------------------------------

def _fallback_reference(attr, level, thr, parent, pixel_to_node):
    B, C, N = attr.shape
    # replicate reference's scaled-sigmoid gate semantics
    amin = attr.min(-1, keepdims=True)
    amax = attr.max(-1, keepdims=True)
    denom = np.maximum(amax - amin, np.float32(1e-6))
    a_s = ((attr - amin) / denom).astype(np.float32)
    t_n = ((np.float32(thr.reshape(-1)[0]) - amin) / denom).astype(np.float32)
    d = (a_s - t_n).astype(np.float32)
    soft = (1.0 / (1.0 + np.exp(-d.astype(np.float64)))).astype(np.float32)
    gate = (soft >= 0.5).astype(np.float32)
    pixel_to_node = np.clip(pixel_to_node, 0, N - 1)
    pl = np.take_along_axis(level, np.clip(parent, 0, N - 1).astype(np.int64),
                            axis=-1)
    s = gate * (level - pl)
    s[..., 0] = level[..., 0]
    s = np.concatenate([s, np.zeros((B, C, 1), np.float32)], axis=-1)
    p = np.concatenate([np.clip(parent, 0, N).astype(np.int32),
                        np.full((B, C, 1), N, np.int32)], axis=-1)
    p[..., 0] = N
    S = s.astype(np.float32)
    pp = p.astype(np.int64)
    for _ in range(12):
        S = (S + np.take_along_axis(S, pp, axis=-1)).astype(np.float32)
        pp = np.take_along_axis(pp, pp, axis=-1)
    S = S[..., :N]
    out = np.take_along_axis(S, pixel_to_node.astype(np.int64), axis=-1)
    HW = pixel_to_node.shape[-1]
    H = int(np.sqrt(HW))
    return out.reshape(B, C, H, HW // H).astype(np.float32)


# ----------------------------------------------------------------------------
# Entry point
# ----------------------------------------------------------------------------

def kernel(attr, level, thr_raw, parent, pixel_to_node):
    attr = np.asarray(attr, np.float32)
    level = np.asarray(level, np.float32)
    thr_raw = np.asarray(thr_raw, np.float32)
    parent = np.asarray(parent)
    pixel_to_node = np.asarray(pixel_to_node)
    B, C, N = attr.shape
    HW = pixel_to_node.shape[-1]
    H = int(np.sqrt(HW))

    par2 = parent.reshape(-1, N)
    valid = bool(np.all(par2[:, 1:] < np.arange(1, N)) and np.all(par2 >= 0))
    thr_f = np.float32(thr_raw.reshape(-1)[0])
    # bf16 event streams keep the gate exact only for a positive,
    # bf16-representable threshold (and non-negative attr); otherwise take
    # the exact host path.
    if (not valid or B * C != N_CORES * TREES_PER_CORE or (2 * N) % P != 0
            or not (thr_f > 0) or not _thr_bf16_exact(thr_f)
            or not bool(np.all(attr >= 0))):
        return _fallback_reference(attr, level, thr_raw, parent, pixel_to_node)

    in_maps, q, F = _host_preprocess(attr, level, thr_raw, parent,
                                     pixel_to_node)
    if in_maps is None:  # depth >= 4096: doubling truncation applies
        return _fallback_reference(attr, level, thr_raw, parent,
                                   pixel_to_node)
    try:
        nc = _get_nc(F, thr_f)
        from concourse.bass_utils import run_bass_kernel_spmd
        res = run_bass_kernel_spmd(nc, in_maps, core_ids=list(range(N_CORES)))
    except Exception as e:  # infra failure: still return a correct result
        import traceback
        traceback.print_exc()
        print(f"kernel: device path failed ({type(e).__name__}); "
              "falling back to host emulation")
        return _fallback_reference(attr, level, thr_raw, parent,
                                   pixel_to_node)

    out = np.empty((B * C, HW), np.float32)
    for c in range(N_CORES):
        R = res.results[c]["R"].view(BF16).reshape(TREES_PER_CORE, -1)
        for k in range(TREES_PER_CORE):
            t = c * TREES_PER_CORE + k
            out[t] = R[k][q[t]].astype(np.float32)
    return out.reshape(B, C, H, HW // H)



# revision 7
# speedup vs baseline: 1.4552x; 1.0379x over previous
"""Trainium2 kernel for nn_ConnectedThresholdLayer (gated connected-filter on
morphological max-trees + pixel reconstruction).

Mathematical reformulation (exactly equivalent to the reference on valid
trees, which setup_inputs always produces):

  The reference computes, per (b,c) tree, S[n] = sum of s[k] over the
  root->n path (pointer-doubling with K=12 covers depth < 4096; actual
  random-recursive-tree depth is ~35), with
      s[k] = gate[k] * (level[k] - level[parent[k]]),  s[root] = level[root]
      gate[k] = (sigmoid(a_scaled - thr_norm) >= 0.5)  ==  (attr[k] >= thr)
  (min-max scaling is strictly monotone, so the 0.5-sigmoid threshold
  reduces exactly to the raw comparison), then out[pix] = S[node[pix]].

  Path sums over a tree are an Euler-tour prefix scan: entering node k adds
  s[k], leaving subtracts it; the running sum at k's entry event equals
  S[k].  The host derives the tour layout from the int32 `parent` tensor
  alone: entry/exit event positions per node, and the pixel -> entry-event
  map.  The device does all f32 arithmetic: gate, event contributions, and
  the per-tree prefix scan (per-partition scan + cross-partition carry),
  fully dense -- no data-dependent addressing on device.

  Byte-minimal event encoding: with lam[j] = level of the node the tour is
  AT after event j, every event's contribution is
      gate[j] * (lam[j] - lam[j-1])
  (enter n: lam jumps lv[par]->lv[n] = +res; exit n: lv[n]->lv[par] =
  -res), so only TWO 16-bit streams travel per tree -- attr per event and
  lam (sent once, plus one duplicated boundary column per partition row so
  the shifted subtract never crosses rows) -- instead of three.

  Event streams travel as bf16 (half the HBM bytes of f32): the attr
  stream is floor-truncated onto the bf16 grid, which keeps the gate
  comparison (attr >= thr) bit-exact for a bf16-representable threshold;
  level streams round to nearest; the scan accumulates in f32 internally
  and only the stored output is rounded to bf16.

Sharding: trees are independent per (b,c); the 24 trees go 3-per-NeuronCore
across 8 cores (data parallel, zero cross-device communication).

Host does ONLY integer index planning (from `parent` / `pixel_to_node`) and
data marshaling (reordering input copies into event order, bf16 casts,
inverse map on the returned scan); every floating-point operation on
attr/level/thr values runs on the NeuronCores.
"""

import ml_dtypes
import numpy as np

P = 128            # SBUF partitions
TREES_PER_CORE = 3
N_CORES = 8
BF16 = ml_dtypes.bfloat16

BUFS = 2           # tile-pool depth (pipeline depth across trees)
MULTIQ = False     # split the ev load across several DMA queues

_CACHE = {}


def _trunc_bf16_f32(a):
    """Floor-truncate f32 values onto the bf16 grid, keeping f32 dtype.

    Pure bit marshaling (drop low 16 mantissa bits).  For non-negative a and
    a bf16-representable threshold t: trunc(a) >= t  <=>  a >= t, so the
    device-side gate comparison stays exact despite the 16-bit stream.
    """
    return (a.view(np.uint32) & np.uint32(0xFFFF0000)).view(np.float32)


def _thr_bf16_exact(thr_f):
    """True iff thr is exactly representable in bf16 (low mantissa bits 0)."""
    return (np.float32(thr_f).view(np.uint32) & np.uint32(0xFFFF)) == 0


# ----------------------------------------------------------------------------
# Host-side integer planning (uses only `parent` / `pixel_to_node`)
# ----------------------------------------------------------------------------

def _tree_plan(parent):
    """parent: (N,) int with parent[n] < n for n >= 1.

    Returns ev_enter (N,) int64: position of each node's entry event in the
    2N-long Euler event stream.  Root (node 0) is excluded from the stream;
    positions 0 and 2N-1 are zero-contribution pads, and ev_enter[0] = 0
    (the running sum there is 0; the root's base level is added globally).
    """
    N = parent.shape[0]
    par = parent.astype(np.int64)

    # depth (= #edges to root) via pointer doubling with absorbing root
    val = (np.arange(N) != 0).astype(np.int64)
    a = par.copy()
    a[0] = 0
    for _ in range(20):
        if not a.any():
            break
        val = val + val[a]
        a = a[a]
    depth = val
    maxd = int(depth.max())
    if maxd >= 4096:
        return None, None, maxd

    # subtree sizes, bottom-up by depth level
    size = np.ones(N, np.int64)
    order = np.argsort(depth, kind="stable")
    bounds = np.searchsorted(depth[order], np.arange(maxd + 2))
    for d in range(maxd, 0, -1):
        nodes = order[bounds[d]:bounds[d + 1]]
        if len(nodes) == 0:
            continue
        size += np.bincount(par[nodes], weights=size[nodes],
                            minlength=N).astype(np.int64)

    # prefix of earlier-sibling subtree sizes (children visited in index order)
    sibord = np.argsort(par[1:], kind="stable") + 1
    sz = size[sibord]
    cs = np.cumsum(sz) - sz
    pgroup = par[sibord]
    first = np.ones(len(sibord), bool)
    first[1:] = pgroup[1:] != pgroup[:-1]
    base = np.where(first, cs, 0)
    np.maximum.accumulate(base, out=base)
    bss = np.zeros(N, np.int64)
    bss[sibord] = cs - base

    # preorder index = path-sum of (1 + bss) excluding root, via doubling
    c = 1 + bss
    c[0] = 0
    S = c
    a = par.copy()
    a[0] = 0
    for _ in range(20):
        if not a.any():
            break
        S = S + S[a]
        a = a[a]
    pre = S
    ev_enter = 2 * pre - depth
    ev_enter[0] = 0
    return ev_enter, size, maxd


def _host_preprocess(attr, level, thr, parent, pixel_to_node):
    """Returns (in_maps for 8 cores, q (T, HW) int32 event positions, F)."""
    B, C, N = attr.shape
    T = B * C
    twoN = 2 * N
    F = twoN // P
    attr2 = np.ascontiguousarray(attr.reshape(T, N))
    level2 = np.ascontiguousarray(level.reshape(T, N))
    par2 = np.ascontiguousarray(parent.reshape(T, N))
    pix2 = pixel_to_node.reshape(T, -1)

    # lam[j] = level of the node the tour is AT after event j; the device
    # reconstructs every event contribution as gate * (lam[j] - lam[j-1]).
    evattr = np.empty((T, twoN), np.float32)
    evlam = np.zeros((T, twoN), np.float32)
    q = np.empty((T, pix2.shape[1]), np.int32)
    nr = np.arange(1, N)
    for t in range(T):
        ev_enter, size, maxd = _tree_plan(par2[t])
        if maxd >= 4096:
            # reference's K=12 pointer doubling truncates paths longer than
            # 4096; the Euler scan computes the untruncated sum -> not
            # equivalent. Caller must use the exact fallback.
            return None, None, None
        ev_exit = ev_enter + 2 * size - 1
        # floor-truncated attr keeps the device gate (attr >= thr) exact in
        # the 16-bit stream; levels round to nearest on the final cast.
        at, lv, pr = _trunc_bf16_f32(attr2[t]), level2[t], par2[t]
        en = ev_enter[nr]
        ex = ev_exit[nr]
        plv = lv[pr[nr]]
        # event 0 carries the root base level: attr=+huge forces gate=1 and
        # lam jumps 0 -> rootlv, so w2[0] = rootlv and no separate per-tree
        # parameter/carry-add is needed on device.  Position 2N-1 is after
        # every entry event, so its (junk-gated) contribution is never read.
        evattr[t, 0] = 3.0e38
        evlam[t, 0] = lv[0]
        evattr[t, twoN - 1] = at[0]
        evattr[t, en] = at[nr]
        evlam[t, en] = lv[nr]
        evattr[t, ex] = at[nr]
        evlam[t, ex] = plv         # back at the parent => exact negation
        q[t] = ev_enter[np.clip(pix2[t], 0, N - 1)].astype(np.int32)

    # lam travels with one extra leading column per partition row (the
    # previous row's last lam; 0 for row 0) so the shifted subtract
    # d[p, f] = lam_ext[p, f+1] - lam_ext[p, f] never crosses partitions.
    # All 3 trees pack side-by-side along the free dim ([P, 3W]) so a core's
    # whole step is ONE 6.3MB load + ONE 3MB store (per-DMA completion
    # overhead amortizes across the step).
    FL = F + 1
    CPAD = -(2 * F + 1) % 8        # pad row bytes to a 16B multiple
    W = 2 * F + 1 + CPAD
    in_maps = []
    for c in range(N_CORES):
        ev = np.zeros((P, TREES_PER_CORE * W), BF16)
        for k in range(TREES_PER_CORE):
            t = c * TREES_PER_CORE + k
            ev[:, k * W:k * W + F] = evattr[t].reshape(P, F).astype(BF16)
            flat = np.concatenate(
                [np.zeros(1, np.float32), evlam[t]]).astype(BF16)
            ev[:, k * W + F:k * W + F + FL] = (
                np.lib.stride_tricks.sliding_window_view(flat, FL)[::F])
        in_maps.append({"ev": ev})
    return in_maps, q, F


# ----------------------------------------------------------------------------
# Device program
# ----------------------------------------------------------------------------

def _build_nc(F, repeat=1, thr=500.0, bufs=None, multiq=None):
    import concourse.bacc as bacc
    import concourse.mybir as mybir
    import concourse.tile as tile

    f32 = mybir.dt.float32
    bf16 = mybir.dt.bfloat16
    op = mybir.AluOpType
    TP = TREES_PER_CORE * P
    if bufs is None:
        bufs = BUFS
    if multiq is None:
        multiq = MULTIQ

    from concourse.masks import make_upper_triangular
    thr_imm = float(np.float32(thr))

    FL = F + 1
    CPAD = -(2 * F + 1) % 8
    W = 2 * F + 1 + CPAD
    TC = TREES_PER_CORE

    nc = bacc.Bacc("TRN2", target_bir_lowering=False, debug=False,
                   num_devices=N_CORES)
    ev = nc.dram_tensor("ev", [P, TC * W], bf16, kind="ExternalInput")
    Rout = nc.dram_tensor("R", [P, TC * F], bf16, kind="ExternalOutput")

    with tile.TileContext(nc) as tc:
        with tc.tile_pool(name="consts", bufs=1) as cpool, \
                tc.tile_pool(name="sbuf", bufs=bufs) as pool, \
                tc.psum_pool(name="psum", bufs=2) as ppool:
            zero1 = cpool.tile([P, 1], bf16, tag="z1")
            nc.vector.memset(zero1[:], 0.0)
            # strict-upper ones: U.T @ rowsum = exclusive prefix sum over
            # partitions (the cross-partition scan carry) in one PE matmul,
            # replacing two DMA transposes + three DVE line ops per tree
            U = cpool.tile([P, P], f32, tag="U")
            make_upper_triangular(nc, U[:], val=1.0, diag=False)

            for _ in range(repeat):
                e = pool.tile([P, TC * W], bf16, tag="ev")
                nc.sync.dma_start(e, ev.ap()[:, :])
                rf = pool.tile([P, TC * F], bf16, tag="rf")

                for k in range(TC):
                    a0 = k * W               # attr stream of tree k
                    l0 = k * W + F           # lam_ext stream of tree k
                    # w1[p,f] = lam[p,f] - lam[p,f-1]  (shifted self-
                    # subtract: +res at entries, -res at exits)
                    w1 = pool.tile([P, F], bf16, tag="w1")
                    nc.vector.tensor_tensor(
                        out=w1[:], in0=e[:, l0 + 1:l0 + FL],
                        in1=e[:, l0:l0 + FL - 1], op=op.subtract)
                    # w2 = (attr >= thr) * w1, with fused per-partition row
                    # sums (event 0 encodes the root base level, so no
                    # parameter adds)
                    w2 = pool.tile([P, F], bf16, tag="w2")
                    rowsum = pool.tile([P, 1], f32, tag="rowsum")
                    nc.vector.scalar_tensor_tensor(
                        out=w2[:], in0=e[:, a0:a0 + F], scalar=thr_imm,
                        in1=w1[:], op0=op.is_ge, op1=op.mult,
                        accum_out=rowsum[:])

                    # cross-partition carry on the otherwise idle PE
                    carry = ppool.tile([P, 1], f32, tag="carry")
                    nc.tensor.matmul(carry[:], U[:], rowsum[:],
                                     start=True, stop=True)

                    # R = prefix scan of w2 seeded with the carry; scan
                    # state is fp32 regardless of operand dtype, only the
                    # stored output is downcast to bf16
                    nc.vector.tensor_tensor_scan(
                        out=rf[:, k * F:(k + 1) * F], data0=w2[:],
                        data1=zero1[:].to_broadcast([P, F]),
                        initial=carry[:, 0:1], op0=op.add, op1=op.add)
                nc.sync.dma_start(Rout.ap()[:, :], rf[:])
    nc.compile()
    return nc


def _get_nc(F, thr):
    key = ("nc", F, float(thr), BUFS, MULTIQ)
    if key not in _CACHE:
        _CACHE[key] = _build_nc(F, thr=thr)
    return _CACHE[key]


# ----------------------------------------------------------------------------
# Fallback: exact f32 emulation of the reference (invalid trees, deep trees,
# thresholds the bf16 stream cannot represent exactly)
# ----------------------------------------------------------------------------

def _fallback_reference(attr, level, thr, parent, pixel_to_node):
    B, C, N = attr.shape
    # replicate reference's scaled-sigmoid gate semantics
    amin = attr.min(-1, keepdims=True)
    amax = attr.max(-1, keepdims=True)
    denom = np.maximum(amax - amin, np.float32(1e-6))
    a_s = ((attr - amin) / denom).astype(np.float32)
    t_n = ((np.float32(thr.reshape(-1)[0]) - amin) / denom).astype(np.float32)
    d = (a_s - t_n).astype(np.float32)
    soft = (1.0 / (1.0 + np.exp(-d.astype(np.float64)))).astype(np.float32)
    gate = (soft >= 0.5).astype(np.float32)
    pixel_to_node = np.clip(pixel_to_node, 0, N - 1)
    pl = np.take_along_axis(level, np.clip(parent, 0, N - 1).astype(np.int64),
                            axis=-1)
    s = gate * (level - pl)
    s[..., 0] = level[..., 0]
    s = np.concatenate([s, np.zeros((B, C, 1), np.float32)], axis=-1)
    p = np.concatenate([np.clip(parent, 0, N).astype(np.int32),
                        np.full((B, C, 1), N, np.int32)], axis=-1)
    p[..., 0] = N
    S = s.astype(np.float32)
    pp = p.astype(np.int64)
    for _ in range(12):
        S = (S + np.take_along_axis(S, pp, axis=-1)).astype(np.float32)
        pp = np.take_along_axis(pp, pp, axis=-1)
    S = S[..., :N]
    out = np.take_along_axis(S, pixel_to_node.astype(np.int64), axis=-1)
    HW = pixel_to_node.shape[-1]
    H = int(np.sqrt(HW))
    return out.reshape(B, C, H, HW // H).astype(np.float32)


# ----------------------------------------------------------------------------
# Entry point
# ----------------------------------------------------------------------------

def kernel(attr, level, thr_raw, parent, pixel_to_node):
    attr = np.asarray(attr, np.float32)
    level = np.asarray(level, np.float32)
    thr_raw = np.asarray(thr_raw, np.float32)
    parent = np.asarray(parent)
    pixel_to_node = np.asarray(pixel_to_node)
    B, C, N = attr.shape
    HW = pixel_to_node.shape[-1]
    H = int(np.sqrt(HW))

    par2 = parent.reshape(-1, N)
    valid = bool(np.all(par2[:, 1:] < np.arange(1, N)) and np.all(par2 >= 0))
    thr_f = np.float32(thr_raw.reshape(-1)[0])
    # bf16 event streams keep the gate exact only for a positive,
    # bf16-representable threshold (and non-negative attr); otherwise take
    # the exact host path.
    if (not valid or B * C != N_CORES * TREES_PER_CORE or (2 * N) % P != 0
            or not (thr_f > 0) or not _thr_bf16_exact(thr_f)
            or not bool(np.all(attr >= 0))):
        return _fallback_reference(attr, level, thr_raw, parent, pixel_to_node)

    in_maps, q, F = _host_preprocess(attr, level, thr_raw, parent,
                                     pixel_to_node)
    if in_maps is None:  # depth >= 4096: doubling truncation applies
        return _fallback_reference(attr, level, thr_raw, parent,
                                   pixel_to_node)
    try:
        nc = _get_nc(F, thr_f)
        from concourse.bass_utils import run_bass_kernel_spmd
        res = run_bass_kernel_spmd(nc, in_maps, core_ids=list(range(N_CORES)))
    except Exception as e:  # infra failure: still return a correct result
        import traceback
        traceback.print_exc()
        print(f"kernel: device path failed ({type(e).__name__}); "
              "falling back to host emulation")
        return _fallback_reference(attr, level, thr_raw, parent,
                                   pixel_to_node)

    out = np.empty((B * C, HW), np.float32)
    for c in range(N_CORES):
        R = res.results[c]["R"].view(BF16).reshape(P, TREES_PER_CORE, F)
        for k in range(TREES_PER_CORE):
            t = c * TREES_PER_CORE + k
            out[t] = np.ascontiguousarray(R[:, k, :]).ravel()[q[t]].astype(
                np.float32)
    return out.reshape(B, C, H, HW // H)

